# revision 80
# baseline (speedup 1.0000x reference)
"""BiDAF attention-flow kernel for Trainium2 (8 NeuronCores, data-parallel over batch).

bf16 redesign. Host pre-tiles inputs into bf16 layouts (h token-major, hT
feature-major with a baked-in ones row for the uw bias-fold, u2 with ones
column for the softmax denominator, uT for the lhsT build); the device
computes only the three derived output blocks (c2q, h*c2q, h*q2c) in bf16,
and the host assembles g = [h | c2q | h*c2q | h*q2c] in f32 (block 0 is the
original f32 input h; the rest upcast from bf16). rel-err budget is 2e-2 --
bf16 rounding (~0.4%) is far inside it.

Per-core per-batch math (T=2048, J=256, D=200):
  s[t,j] = h.w_h[t] + u.w_u[j] + (h*w_hu).u[t,j]   (+bias, uniform shift that
           cancels in both softmaxes; b_* are zeros anyway)
  a      = softmax_j(s); c2q = a @ u
  m[t]   = max_j s; beta = softmax_t(m); q2c = beta @ h

Layout: S^T = umodT @ hT with j on partitions, so the exp'd scores feed the
c2q matmul directly as lhsT. uw (u.w_u) rides as an extra lhsT row (row 96 of
chunk 1, an allowed compute partition start) against a ones row baked into hT
-- the exp needs no bias operand. hw (h.w_h) accumulates both d-chunks into
one psum column per tile (back-to-back pair, never interleaved in its bank)
and folds into the T-softmax weights multiplicatively (em = rowmax_exp *
exp(hw)); the per-t shift cancels in softmax_j. Block 3 is produced
FEATURE-major (g3T[d,t] = hT[d,t]*q2c[d]*rts) by one fused two-scalar DVE
tensor_scalar per d-chunk -- no partition broadcast or h*qb muls.

Everything dense is bf16 (PE 1 cyc/row for matmul and transpose at any N; DVE
2x on packed bf16 SBUF, 4x for tensor_scalar), accumulation in f32 PSUM.
j-max: DVE max of the two exp'd halves (Pool lacks the TT-max opcode and has
no PSUM port) -> 4 PE transposes into one bf16 psum tile -> one batched DVE
reduce_max per group. c2q matmuls use N=201 (200 dims + ones denominator
column). Softmax normalize: one DVE tensor_tensor per group with the
reciprocals broadcast along d via a stride-0 AP. h*c2q: one batched Pool mul
per group. em weights + the 32 q2c partial matmuls run at the batch tail.

Hardware rules honored (walrus verifier): GPSIMD touches SBUF only; at most
one PSUM operand per TensorTensor; compute APs start at partition 0/32/64/96;
matmul outputs are f32 within a single psum bank. Tiny tiles read via the
tensor_scalar scalar slot live in a bufs=8 pool -- region reuse across the
batch rotation races the scalar-slot read otherwise (observed in exec sim).

DMA queues (each transfer occupies its issuing engine in the cost model):
SP: gb stores + hT/u2 loads; Pool: g3 stores, h first half, uT; ACT: h second
half. All DRAM runs are >= 512 B to dodge the narrow-descriptor penalty.
Cost-model simulated time: ~93.4 us/core (engine busy: DVE 79, SP 70,
Pool 64, PE 57, ACT 55); hardware-validated rel err ~7.7e-3.
"""

import sys

sys.path.insert(0, "/opt/trn_rl_repo")

import numpy as np
import ml_dtypes

import concourse.bass as bass
import concourse.bacc as bacc
import concourse.mybir as mybir
import concourse.tile as tile

B, T, J, D = 64, 2048, 256, 200
N_CORES = 8
BL = B // N_CORES          # batches per core
NT = T // 128              # 16 t-tiles per batch
NG = NT // 4               # 4 groups of 512 tokens
F32 = mybir.dt.float32
BF16 = mybir.dt.bfloat16
AX = mybir.AxisListType
AF = mybir.ActivationFunctionType
BFNP = ml_dtypes.bfloat16

_cache = {}
DEBUG_DUMPS = False
GW = 4            # tiles (128 tokens) per pipeline group
HC2Q_MOD = 1      # 0 all-DVE, 1 all-Pool, 2 alternate
H2_ENG = "scalar"    # engine for the second half of the h load
G30_ENG = "gpsimd"   # engine for g3 chunk0 store
G31_ENG = "gpsimd"   # engine for g3 chunk1 store
NORM_ACT_EVERY = 0   # every Nth group's normalize runs per-tile on ACT


def build_nc():
    nc = bacc.Bacc()
    h_d = nc.dram_tensor("hD", [BL, 128, NT * D], BF16, kind="ExternalInput")
    hT_d = nc.dram_tensor("hTD", [BL, 2, 128, T], BF16, kind="ExternalInput")
    u2_d = nc.dram_tensor("u2D", [BL, 128, 2, 208], BF16, kind="ExternalInput")
    uT_d = nc.dram_tensor("uTD", [BL, 2, 128, 256], BF16, kind="ExternalInput")
    wb_d = nc.dram_tensor("wbf", [128, 6], BF16, kind="ExternalInput")
    wf_d = nc.dram_tensor("wf", [128, 2], F32, kind="ExternalInput")
    gb_d = nc.dram_tensor("gbD", [BL, 128, NT, 2 * D], BF16, kind="ExternalOutput")
    # block 3 is produced feature-major: g3T[d,t] = hT[d,t] * q2c[d] / sum(em)
    g3_d = nc.dram_tensor("g3D", [BL, 2, 128, T], BF16, kind="ExternalOutput")
    if DEBUG_DUMPS:
        em_d = nc.dram_tensor("emD", [BL, 128, NT], BF16, kind="ExternalOutput")
        mc_d = nc.dram_tensor("mcD", [BL, 128, NT], F32, kind="ExternalOutput")
        pq_d = nc.dram_tensor("pqD", [BL, 128, 80], F32, kind="ExternalOutput")
        qt_d = nc.dram_tensor("qtD", [BL, 128, 2], F32, kind="ExternalOutput")
        rt_d = nc.dram_tensor("rtD", [BL, 128, 1], F32, kind="ExternalOutput")

    with tile.TileContext(nc) as tc:
        with (
            tc.tile_pool(name="sing", bufs=1) as sing,
            tc.tile_pool(name="hp", bufs=3) as hp,
            tc.tile_pool(name="htp", bufs=3) as htp,
            tc.tile_pool(name="upool", bufs=3) as upool,
            tc.tile_pool(name="lhsu", bufs=3) as lhsu,
            tc.tile_pool(name="ptp", bufs=2) as ptp,
            tc.tile_pool(name="ggp", bufs=3) as ggp,
            tc.tile_pool(name="g3p", bufs=2) as g3p,
            tc.tile_pool(name="mxp", bufs=3) as mxp,
            tc.tile_pool(name="smalls", bufs=2) as smalls,
            tc.tile_pool(name="qtp", bufs=8) as qtp,
            tc.tile_pool(name="rcp", bufs=4) as rcp,
            tc.tile_pool(name="pp", bufs=1, space="PSUM") as pp,
            tc.tile_pool(name="pcq", bufs=1, space="PSUM") as pcq,
            tc.tile_pool(name="pmx", bufs=2, space="PSUM") as pmx,
            tc.tile_pool(name="pqp", bufs=2, space="PSUM") as pqp,
        ):
            ident_bf_d = nc.inline_tensor(
                np.eye(128, dtype=BFNP), name="ident_bf"
            )
            ident_bf = sing.tile([128, 128], BF16)
            nc.sync.dma_start(out=ident_bf, in_=ident_bf_d[:, :])
            wb = sing.tile([128, 6], BF16)   # wh0 wh1 wu0 wu1 ones -
            nc.sync.dma_start(out=wb, in_=wb_d[:, :])
            wf = sing.tile([128, 2], F32)    # whu chunks (tensor_scalar scalar)
            nc.sync.dma_start(out=wf, in_=wf_d[:, :])

            def load_batch(b):
                hsb = hp.tile([128, NT * D], BF16, tag="h", name=f"h{b}")
                hT = htp.tile([128, 2, T], BF16, tag="hT", name=f"hT{b}")
                u2 = upool.tile([128, 2, 208], BF16, tag="u2", name=f"u2{b}")
                lhsU = lhsu.tile([128, 2, 256], BF16, tag="l", name=f"lhsU_{b}")
                lhsU0, lhsU1 = lhsU[:, 0, :], lhsU[:, 1, :]
                nc.gpsimd.dma_start(out=hsb[:, 0 : NT * D // 2], in_=h_d[b, :, 0 : NT * D // 2])
                getattr(nc, H2_ENG).dma_start(out=hsb[:, NT * D // 2 :], in_=h_d[b, :, NT * D // 2 :])
                nc.sync.dma_start(out=hT, in_=hT_d[b].rearrange("c p t -> p c t"))
                nc.sync.dma_start(out=u2, in_=u2_d[b])
                # rows 72:128 of chunk 1 host-zeroed; row 96 gets uw at prep
                nc.gpsimd.dma_start(out=lhsU, in_=uT_d[b].rearrange("c p j -> p c j"))
                return hsb, hT, u2, lhsU

            def prep_batch(b, lhsU):
                lhsU0 = lhsU[:, 0, :]
                # pq columns (single-shot matmuls only): 0:16 q2c d<128,
                # 16:32 q2c d>=128, 32:48 hw chunk0, 48:64 hw chunk1,
                # 64:80 em-total row, 256:512 uw scratch (batch-prep time)
                pq = pqp.tile([128, 512], F32, tag="q", name=f"pq{b}")
                # uw[j] = u . w_u from the raw uT, lands in lhsU1 row 72 where
                # it meets the ones row baked into hT chunk 1.
                nc.tensor.matmul(
                    pq[0:1, 256:512], wb[:, 2:3], lhsU0, start=True, stop=False
                )
                nc.tensor.matmul(
                    pq[0:1, 256:512], wb[0:72, 3:4], lhsU[0:72, 1, :],
                    start=False, stop=True,
                )
                nc.scalar.copy(lhsU[96:97, 1, :], pq[0:1, 256:512])
                # fold w_hu into uT (in place; rows 0:72 only on chunk 1)
                nc.gpsimd.tensor_scalar_mul(lhsU0, lhsU0, wf[:, 0:1])
                nc.gpsimd.tensor_scalar_mul(lhsU[0:72, 1, :], lhsU[0:72, 1, :], wf[0:72, 1:2])
                return pq

            pending = load_batch(0)
            pending = pending + (prep_batch(0, pending[3]),)
            for b in range(BL):
                hsb, hT, u2, lhsU, pq = pending
                lhsU0, lhsU1 = lhsU[:, 0, :], lhsU[:, 1, :]
                if b + 1 < BL:
                    nxt = load_batch(b + 1)
                    pending = nxt + (prep_batch(b + 1, nxt[3]),)

                mcol = smalls.tile([128, NT], F32, tag="mcol")
                em = smalls.tile([128, NT], BF16, tag="em")

                for gi in range(NT // GW):
                    gw0 = GW * 128 * gi          # token offset of this group
                    tsl = slice(gw0, gw0 + GW * 128)
                    # per-jc S^T tiles with per-jc exps: exp(jc) frees its bank
                    # while the other half's matmuls still run
                    pT = []
                    for jc in range(2):
                        pST = pp.tile([128, GW * 128], F32, tag=f"ST{jc}")
                        nc.tensor.matmul(
                            pST, lhsU0[:, 128 * jc : 128 * (jc + 1)],
                            hT[:, 0, tsl], start=True, stop=False,
                        )
                        nc.tensor.matmul(
                            pST, lhsU[0:97, 1, 128 * jc : 128 * (jc + 1)],
                            hT[0:97, 1, tsl], start=False, stop=True,
                        )
                        pT_sb = ptp.tile([128, GW * 128], BF16, tag=f"pT{jc}")
                        nc.scalar.activation(pT_sb, pST, AF.Exp)
                        pT.append(pT_sb)

                    # j-max: halve on Pool in SBUF, transpose, one batched reduce
                    pmax = mxp.tile([128, GW * 128], BF16, tag="pmax")
                    nc.vector.tensor_tensor(
                        pmax, pT[0], pT[1], op=mybir.AluOpType.max
                    )
                    pmg = pmx.tile([128, GW, 128], BF16, tag="mx", name="pmg")
                    for k in range(GW):
                        nc.tensor.transpose(
                            pmg[:, k, :], pmax[:, 128 * k : 128 * (k + 1)], ident_bf
                        )
                    slg = slice(GW * gi, GW * gi + GW)
                    nc.vector.reduce_max(mcol[:, slg], pmg, axis=AX.X)

                    # c2q + denominator for the whole group into one psum tile
                    pcg = pcq.tile([128, GW, 256], F32, tag="cq", name="pcg")
                    for k in range(GW):
                        i = GW * gi + k
                        sl = slice(128 * k, 128 * (k + 1))
                        nc.tensor.matmul(
                            pcg[:, k, 0:201], pT[0][:, sl], u2[:, 0, 0:201],
                            start=True, stop=False,
                        )
                        nc.tensor.matmul(
                            pcg[:, k, 0:201], pT[1][:, sl], u2[:, 1, 0:201],
                            start=False, stop=True,
                        )
                        # hw[t] = h.w_h accumulated over both d-chunks; the
                        # pair is emitted back-to-back so the accumulation
                        # group never interleaves with another in this bank
                        nc.tensor.matmul(
                            pq[:, 32 + i : 33 + i], hT[:, 0, 128 * i : 128 * i + 128],
                            wb[:, 0:1], start=True, stop=False,
                        )
                        nc.tensor.matmul(
                            pq[:, 32 + i : 33 + i],
                            hT[0:72, 1, 128 * i : 128 * i + 128],
                            wb[0:72, 1:2], start=False, stop=True,
                        )

                    # one reciprocal for the group's denominators
                    rc = rcp.tile([128, GW], F32, tag="rc")
                    nc.vector.reciprocal(rc, pcg[:, :, 200])
                    gg = ggp.tile([128, GW, 2 * D], BF16, tag="gg")
                    # normalize all GW tiles in one op: rc broadcast along d via
                    # a stride-0 AP (DVE), with an ACT per-tile variant for
                    # load balancing every NORM_ACT_EVERYth group
                    gidx = b * (NT // GW) + gi
                    if NORM_ACT_EVERY and gidx % NORM_ACT_EVERY == NORM_ACT_EVERY - 1:
                        for k in range(GW):
                            nc.scalar.mul(
                                gg[:, k, 0:D], pcg[:, k, 0:D], mul=rc[:, k : k + 1]
                            )
                    else:
                        rca = rc[:, :]
                        rcb = bass.AP(
                            tensor=rca.tensor, offset=rca.offset,
                            ap=[*rca.ap, [0, D]],
                        )
                        nc.vector.tensor_tensor(
                            gg[:, :, 0:D], pcg[:, :, 0:D], rcb,
                            op=mybir.AluOpType.mult,
                        )
                    # h*c2q for the whole group in one op
                    hsb_g = hsb[:, D * GW * gi : D * GW * (gi + 1)].rearrange(
                        "p (k d) -> p k d", d=D
                    )
                    if HC2Q_MOD == 1 or (HC2Q_MOD == 2 and gidx % 2 == 0):
                        nc.gpsimd.tensor_mul(gg[:, :, D : 2 * D], hsb_g, gg[:, :, 0:D])
                    else:
                        nc.vector.tensor_mul(gg[:, :, D : 2 * D], hsb_g, gg[:, :, 0:D])
                    nc.sync.dma_start(
                        out=gb_d[b, :, GW * gi : GW * gi + GW, :],
                        in_=gg,
                    )

                # ---- batch tail: em weights, q2c, broadcast, final product ----
                # em = rowmax_exp * exp(hw), one exp + one mul
                eh16 = smalls.tile([128, NT], F32, tag="eh16")
                nc.scalar.activation(eh16, pq[:, 32:48], AF.Exp)
                nc.gpsimd.tensor_mul(em, mcol, eh16)
                # q2c partials, one column per tile (no open psum groups)
                for i in range(NT):
                    nc.tensor.matmul(
                        pq[:, i : i + 1], hsb[:, D * i : D * i + 128],
                        em[:, i : i + 1], start=True, stop=True,
                    )
                    nc.tensor.matmul(
                        pq[0:72, 16 + i : 17 + i],
                        hsb[:, D * i + 128 : D * i + D],
                        em[:, i : i + 1], start=True, stop=True,
                    )
                nc.tensor.matmul(
                    pq[0:1, 64:80], wb[:, 4:5], em, start=True, stop=True
                )
                rts = smalls.tile([1, 1], F32, tag="rts")
                nc.vector.reduce_sum(rts, pq[0:1, 64:80], axis=AX.X)
                nc.vector.reciprocal(rts, rts)
                rtsc = qtp.tile([128, 1], F32, tag="rtsc")
                nc.gpsimd.partition_broadcast(rtsc, rts)
                q2cT = qtp.tile([128, 2], F32, tag="q2cT")
                nc.vector.reduce_sum(q2cT[:, 0:1], pq[:, 0:16], axis=AX.X)
                nc.vector.reduce_sum(q2cT[0:72, 1:2], pq[0:72, 16:32], axis=AX.X)
                # block 3 feature-major in one fused op per d-chunk:
                # g3T[d,t] = hT[d,t] * q2c_raw[d] * (1/sum em)
                # chunk1 rows 72:128 of q2cT are never written -- compute and
                # store only the valid 0:72 rows for that chunk
                g3t = g3p.tile([128, 2, T], BF16, tag="g3")
                nc.vector.tensor_scalar(
                    g3t[:, 0, :], hT[:, 0, :], q2cT[:, 0:1], rtsc,
                    op0=mybir.AluOpType.mult, op1=mybir.AluOpType.mult,
                )
                nc.vector.tensor_scalar(
                    g3t[0:72, 1, :], hT[0:72, 1, :], q2cT[0:72, 1:2], rtsc[0:72],
                    op0=mybir.AluOpType.mult, op1=mybir.AluOpType.mult,
                )
                getattr(nc, G30_ENG).dma_start(out=g3_d[b, 0], in_=g3t[:, 0, :])
                g31 = "sync" if b == BL - 1 else G31_ENG
                getattr(nc, g31).dma_start(out=g3_d[b, 1, 0:72], in_=g3t[0:72, 1, :])
                if DEBUG_DUMPS:
                    dcp = smalls.tile([128, 80], F32, tag="dcp")
                    nc.scalar.copy(dcp[:, 0:16], pq[:, 0:16])
                    nc.scalar.copy(dcp[:, 32:64], pq[:, 32:64])
                    nc.gpsimd.memset(dcp[:, 16:32], 0.0)
                    nc.gpsimd.memset(dcp[:, 64:80], 0.0)
                    nc.sync.dma_start(out=em_d[b], in_=em)
                    nc.sync.dma_start(out=mc_d[b], in_=mcol)
                    nc.sync.dma_start(out=pq_d[b], in_=dcp)
                    nc.sync.dma_start(out=qt_d[b], in_=q2cT)
                    nc.sync.dma_start(out=rt_d[b], in_=rtsc)
    nc.finalize()
    return nc


def _make_runner(nc):
    """jit-compiled SPMD runner (cached across kernel() calls)."""
    import jax
    from jax.sharding import Mesh, PartitionSpec
    from jax.experimental.shard_map import shard_map
    from concourse import bass2jax
    from concourse.bass2jax import _bass_exec_p, install_neuronx_cc_hook

    install_neuronx_cc_hook()
    partition_name = nc.partition_id_tensor.name if nc.partition_id_tensor else None
    in_names, out_names, out_avals, zero_outs = [], [], [], []
    for alloc in nc.m.functions[0].allocations:
        if not isinstance(alloc, mybir.MemoryLocationSet):
            continue
        name = alloc.memorylocations[0].name
        if alloc.kind == "ExternalInput":
            if name != partition_name:
                in_names.append(name)
        elif alloc.kind == "ExternalOutput":
            out_names.append(name)
            shape = tuple(alloc.tensor_shape)
            dtype = mybir.dt.np(alloc.dtype)
            out_avals.append(jax.core.ShapedArray(shape, dtype))
            zero_outs.append(np.zeros(shape, dtype))
    all_in_names = in_names + out_names
    if partition_name is not None:
        all_in_names = all_in_names + [partition_name]

    def _body(*args):
        operands = list(args)
        if partition_name is not None:
            operands.append(bass2jax.partition_id_tensor())
        return tuple(
            _bass_exec_p.bind(
                *operands,
                out_avals=tuple(out_avals),
                in_names=tuple(all_in_names),
                out_names=tuple(out_names),
                lowering_input_output_aliases=(),
                sim_require_finite=True,
                sim_require_nnan=True,
                nc=nc,
            )
        )

    devices = jax.devices()[:N_CORES]
    mesh = Mesh(np.asarray(devices), ("core",))
    n_all = len(in_names) + len(out_names)
    sharded = jax.jit(
        shard_map(
            _body, mesh=mesh,
            in_specs=(PartitionSpec("core"),) * n_all,
            out_specs=(PartitionSpec("core"),) * len(out_names),
            check_rep=False,
        ),
        keep_unused=True,
    )
    zeros_cat = [np.zeros((N_CORES * z.shape[0], *z.shape[1:]), z.dtype)
                 for z in zero_outs]
    return sharded, in_names, zeros_cat


def prep_inputs(h, u, w_h, w_u, w_hu):
    """Host-side tiling/casting into the device layouts (full-batch views)."""
    hbf = h.astype(BFNP)
    ubf = u.astype(BFNP)
    # token-major: hD[b, p, n*200+d] = h[b, n*128+p, d]
    hD = np.ascontiguousarray(
        hbf.reshape(B, NT, 128, D).transpose(0, 2, 1, 3)
    ).reshape(B, 128, NT * D)
    # feature-major with ones row at chunk1 row 72
    ht = hbf.transpose(0, 2, 1)  # [B, D, T]
    hTD = np.zeros((B, 2, 128, T), dtype=BFNP)
    hTD[:, 0, :, :] = ht[:, 0:128, :]
    hTD[:, 1, 0:72, :] = ht[:, 128:D, :]
    hTD[:, 1, 96, :] = BFNP(1.0)   # meets the uw row (96) of lhsU1
    # u token-major (j on partitions) + ones column at 200
    u2D = np.zeros((B, 128, 2, 208), dtype=BFNP)
    u2D[:, :, 0, 0:D] = ubf[:, 0:128, :]
    u2D[:, :, 1, 0:D] = ubf[:, 128:J, :]
    u2D[:, :, :, D] = BFNP(1.0)
    # u feature-major for the lhsT build
    ut = ubf.transpose(0, 2, 1)  # [B, D, J]
    uTD = np.zeros((B, 2, 128, 256), dtype=BFNP)
    uTD[:, 0, :, 0:J] = ut[:, 0:128, :]
    uTD[:, 1, 0:72, 0:J] = ut[:, 128:D, :]
    # weight columns
    wbf = np.zeros((128, 6), dtype=BFNP)
    wbf[:, 0] = w_h[0:128].astype(BFNP)
    wbf[0:72, 1] = w_h[128:D].astype(BFNP)
    wbf[:, 2] = w_u[0:128].astype(BFNP)
    wbf[0:72, 3] = w_u[128:D].astype(BFNP)
    wbf[:, 4] = BFNP(1.0)
    wf = np.zeros((128, 2), dtype=np.float32)
    wf[:, 0] = w_hu[0:128]
    wf[0:72, 1] = w_hu[128:D]
    return hD, hTD, u2D, uTD, wbf, wf


def kernel(**inputs):
    h = np.ascontiguousarray(np.asarray(inputs["h"], dtype=np.float32))
    u = np.ascontiguousarray(np.asarray(inputs["u"], dtype=np.float32))
    w_h = np.asarray(inputs["w_h"], dtype=np.float32)
    w_u = np.asarray(inputs["w_u"], dtype=np.float32)
    w_hu = np.asarray(inputs["w_hu"], dtype=np.float32)

    if "runner" not in _cache:
        _cache["nc"] = build_nc()
        _cache["runner"] = _make_runner(_cache["nc"])
    sharded, in_names, zeros_cat = _cache["runner"]

    hD, hTD, u2D, uTD, wbf, wf = prep_inputs(h, u, w_h, w_u, w_hu)
    full = {
        "hD": hD, "hTD": hTD, "u2D": u2D, "uTD": uTD,
        "wbf": np.concatenate([wbf] * N_CORES, axis=0),
        "wf": np.concatenate([wf] * N_CORES, axis=0),
    }
    args = [full[name] for name in in_names] + zeros_cat
    out = sharded(*args)
    outs = {name: np.asarray(o) for name, o in
            zip([n for n in ("gbD", "g3D")], out)}
    gb = outs["gbD"].reshape(B, 128, NT, 2 * D)
    g3 = outs["g3D"].reshape(B, 2, 128, T)

    g = np.empty((B, T, 4 * D), dtype=np.float32)
    g[:, :, 0:D] = h
    g[:, :, D : 3 * D] = (
        gb.transpose(0, 2, 1, 3).reshape(B, T, 2 * D).astype(np.float32)
    )
    blk3 = np.concatenate([g3[:, 0, :, :], g3[:, 1, 0:72, :]], axis=1)
    g[:, :, 3 * D : 4 * D] = blk3.transpose(0, 2, 1).astype(np.float32)
    return g


# revision 86
# speedup vs baseline: 1.0110x; 1.0110x over previous
"""BiDAF attention-flow kernel for Trainium2 (8 NeuronCores, data-parallel over batch).

bf16 redesign. Host pre-tiles inputs into bf16 layouts (h token-major, hT
feature-major with a baked-in ones row for the uw bias-fold, u2 with ones
column for the softmax denominator, uT for the lhsT build); the device
computes only the three derived output blocks (c2q, h*c2q, h*q2c) in bf16,
and the host assembles g = [h | c2q | h*c2q | h*q2c] in f32 (block 0 is the
original f32 input h; the rest upcast from bf16). rel-err budget is 2e-2 --
bf16 rounding (~0.4%) is far inside it.

Per-core per-batch math (T=2048, J=256, D=200):
  s[t,j] = h.w_h[t] + u.w_u[j] + (h*w_hu).u[t,j]   (+bias, uniform shift that
           cancels in both softmaxes; b_* are zeros anyway)
  a      = softmax_j(s); c2q = a @ u
  m[t]   = max_j s; beta = softmax_t(m); q2c = beta @ h

Layout: S^T = umodT @ hT with j on partitions, so the exp'd scores feed the
c2q matmul directly as lhsT. uw (u.w_u) rides as an extra lhsT row (row 96 of
chunk 1, an allowed compute partition start) against a ones row baked into hT
-- the exp needs no bias operand. hw (h.w_h) accumulates both d-chunks into
one psum column per tile (back-to-back pair, never interleaved in its bank)
and folds into the T-softmax weights multiplicatively (em = rowmax_exp *
exp(hw)); the per-t shift cancels in softmax_j. Block 3 is produced
FEATURE-major (g3T[d,t] = hT[d,t]*q2c[d]*rts) by one fused two-scalar DVE
tensor_scalar per d-chunk -- no partition broadcast or h*qb muls.

Everything dense is bf16 (PE 1 cyc/row for matmul and transpose at any N; DVE
2x on packed bf16 SBUF, 4x for tensor_scalar), accumulation in f32 PSUM.
j-max: DVE max of the two exp'd halves (Pool lacks the TT-max opcode and has
no PSUM port) -> 4 PE transposes into one bf16 psum tile -> one batched DVE
reduce_max per group. c2q matmuls use N=201 (200 dims + ones denominator
column). Softmax normalize: one DVE tensor_tensor per group with the
reciprocals broadcast along d via a stride-0 AP. h*c2q: one batched Pool mul
per group. em weights + the 32 q2c partial matmuls run at the batch tail.

Hardware rules honored (walrus verifier): GPSIMD touches SBUF only; at most
one PSUM operand per TensorTensor; compute APs start at partition 0/32/64/96;
matmul outputs are f32 within a single psum bank. Tiny tiles read via the
tensor_scalar scalar slot live in a bufs=8 pool -- region reuse across the
batch rotation races the scalar-slot read otherwise (observed in exec sim).

DMA queues (each transfer occupies its issuing engine in the cost model):
SP: gb stores + hT/u2 loads; Pool: g3 stores, h first half, uT; ACT: h second
half. All DRAM runs are >= 512 B to dodge the narrow-descriptor penalty.
Cost-model simulated time: ~93.4 us/core (engine busy: DVE 79, SP 70,
Pool 64, PE 57, ACT 55); hardware-validated rel err ~7.7e-3.
"""

import sys

sys.path.insert(0, "/opt/trn_rl_repo")

import numpy as np
import ml_dtypes

import concourse.bass as bass
import concourse.bacc as bacc
import concourse.mybir as mybir
import concourse.tile as tile

B, T, J, D = 64, 2048, 256, 200
N_CORES = 8
BL = B // N_CORES          # batches per core
NT = T // 128              # 16 t-tiles per batch
NG = NT // 4               # 4 groups of 512 tokens
F32 = mybir.dt.float32
BF16 = mybir.dt.bfloat16
AX = mybir.AxisListType
AF = mybir.ActivationFunctionType
BFNP = ml_dtypes.bfloat16

_cache = {}
DEBUG_DUMPS = False
GW = 4            # tiles (128 tokens) per pipeline group
HC2Q_MOD = 1      # 0 all-DVE, 1 all-Pool, 2 alternate
H2_ENG = "scalar"    # engine for the second half of the h load
G30_ENG = "gpsimd"   # engine for g3 chunk0 store
G31_ENG = "gpsimd"   # engine for g3 chunk1 store
NORM_ACT_EVERY = 0   # every Nth group's normalize runs per-tile on ACT


def build_nc():
    nc = bacc.Bacc()
    h_d = nc.dram_tensor("hD", [BL, 128, NT * D], BF16, kind="ExternalInput")
    hT_d = nc.dram_tensor("hTD", [BL, 2, 128, T], BF16, kind="ExternalInput")
    u2_d = nc.dram_tensor("u2D", [BL, 128, 2, 208], BF16, kind="ExternalInput")
    uT_d = nc.dram_tensor("uTD", [BL, 2, 128, 256], BF16, kind="ExternalInput")
    wb_d = nc.dram_tensor("wbf", [128, 6], BF16, kind="ExternalInput")
    wf_d = nc.dram_tensor("wf", [128, 2], F32, kind="ExternalInput")
    gb_d = nc.dram_tensor("gbD", [BL, 128, NT, 2 * D], BF16, kind="ExternalOutput")
    # block 3 is produced feature-major: g3T[d,t] = hT[d,t] * q2c[d] / sum(em)
    g3_d = nc.dram_tensor("g3D", [BL, 2, 128, T], BF16, kind="ExternalOutput")
    if DEBUG_DUMPS:
        em_d = nc.dram_tensor("emD", [BL, 128, NT], BF16, kind="ExternalOutput")
        mc_d = nc.dram_tensor("mcD", [BL, 128, NT], F32, kind="ExternalOutput")
        pq_d = nc.dram_tensor("pqD", [BL, 128, 80], F32, kind="ExternalOutput")
        qt_d = nc.dram_tensor("qtD", [BL, 128, 2], F32, kind="ExternalOutput")
        rt_d = nc.dram_tensor("rtD", [BL, 128, 1], F32, kind="ExternalOutput")

    with tile.TileContext(nc) as tc:
        with (
            tc.tile_pool(name="sing", bufs=1) as sing,
            tc.tile_pool(name="hp", bufs=3) as hp,
            tc.tile_pool(name="htp", bufs=3) as htp,
            tc.tile_pool(name="upool", bufs=3) as upool,
            tc.tile_pool(name="lhsu", bufs=3) as lhsu,
            tc.tile_pool(name="ptp", bufs=2) as ptp,
            tc.tile_pool(name="ggp", bufs=3) as ggp,
            tc.tile_pool(name="g3p", bufs=2) as g3p,
            tc.tile_pool(name="mxp", bufs=3) as mxp,
            tc.tile_pool(name="smalls", bufs=2) as smalls,
            tc.tile_pool(name="qtp", bufs=8) as qtp,
            tc.tile_pool(name="rcp", bufs=4) as rcp,
            tc.tile_pool(name="pp", bufs=1, space="PSUM") as pp,
            tc.tile_pool(name="pcq", bufs=1, space="PSUM") as pcq,
            tc.tile_pool(name="pmx", bufs=2, space="PSUM") as pmx,
            tc.tile_pool(name="pqp", bufs=2, space="PSUM") as pqp,
        ):
            ident_bf_d = nc.inline_tensor(
                np.eye(128, dtype=BFNP), name="ident_bf"
            )
            ident_bf = sing.tile([128, 128], BF16)
            nc.sync.dma_start(out=ident_bf, in_=ident_bf_d[:, :])
            wb = sing.tile([128, 6], BF16)   # wh0 wh1 wu0 wu1 ones -
            nc.sync.dma_start(out=wb, in_=wb_d[:, :])
            wf = sing.tile([128, 2], F32)    # whu chunks (tensor_scalar scalar)
            nc.sync.dma_start(out=wf, in_=wf_d[:, :])

            def load_batch(b):
                hsb = hp.tile([128, NT * D], BF16, tag="h", name=f"h{b}")
                hT = htp.tile([128, 2, T], BF16, tag="hT", name=f"hT{b}")
                u2 = upool.tile([128, 2, 208], BF16, tag="u2", name=f"u2{b}")
                lhsU = lhsu.tile([128, 2, 256], BF16, tag="l", name=f"lhsU_{b}")
                lhsU0, lhsU1 = lhsU[:, 0, :], lhsU[:, 1, :]
                nc.gpsimd.dma_start(out=hsb[:, 0 : NT * D // 2], in_=h_d[b, :, 0 : NT * D // 2])
                getattr(nc, H2_ENG).dma_start(out=hsb[:, NT * D // 2 :], in_=h_d[b, :, NT * D // 2 :])
                nc.sync.dma_start(out=hT, in_=hT_d[b].rearrange("c p t -> p c t"))
                nc.sync.dma_start(out=u2, in_=u2_d[b])
                # rows 72:128 of chunk 1 host-zeroed; row 96 gets uw at prep
                nc.gpsimd.dma_start(out=lhsU, in_=uT_d[b].rearrange("c p j -> p c j"))
                return hsb, hT, u2, lhsU

            def prep_batch(b, lhsU):
                lhsU0 = lhsU[:, 0, :]
                # pq columns (single-shot matmuls only): 0:16 q2c d<128,
                # 16:32 q2c d>=128, 32:48 hw chunk0, 48:64 hw chunk1,
                # 64:80 em-total row, 256:512 uw scratch (batch-prep time)
                pq = pqp.tile([128, 512], F32, tag="q", name=f"pq{b}")
                # uw[j] = u . w_u from the raw uT, lands in lhsU1 row 72 where
                # it meets the ones row baked into hT chunk 1.
                nc.tensor.matmul(
                    pq[0:1, 256:512], wb[:, 2:3], lhsU0, start=True, stop=False
                )
                nc.tensor.matmul(
                    pq[0:1, 256:512], wb[0:72, 3:4], lhsU[0:72, 1, :],
                    start=False, stop=True,
                )
                nc.scalar.copy(lhsU[96:97, 1, :], pq[0:1, 256:512])
                # fold w_hu into uT (in place; rows 0:72 only on chunk 1)
                nc.gpsimd.tensor_scalar_mul(lhsU0, lhsU0, wf[:, 0:1])
                nc.gpsimd.tensor_scalar_mul(lhsU[0:72, 1, :], lhsU[0:72, 1, :], wf[0:72, 1:2])
                return pq

            pending = load_batch(0)
            pending = pending + (prep_batch(0, pending[3]),)
            for b in range(BL):
                hsb, hT, u2, lhsU, pq = pending
                lhsU0, lhsU1 = lhsU[:, 0, :], lhsU[:, 1, :]
                if b + 1 < BL:
                    nxt = load_batch(b + 1)
                    pending = nxt + (prep_batch(b + 1, nxt[3]),)

                mcol = smalls.tile([128, NT], F32, tag="mcol")
                em = smalls.tile([128, NT], BF16, tag="em")

                for gi in range(NT // GW):
                    gw0 = GW * 128 * gi          # token offset of this group
                    tsl = slice(gw0, gw0 + GW * 128)
                    # per-jc S^T tiles with per-jc exps: exp(jc) frees its bank
                    # while the other half's matmuls still run
                    if gi % 2 == 0:
                        pTP = ptp.tile([128, 2, 2, GW * 128], BF16, tag="pT")
                    half = gi % 2
                    pT = []
                    for jc in range(2):
                        pST = pp.tile([128, GW * 128], F32, tag=f"ST{jc}")
                        nc.tensor.matmul(
                            pST, lhsU0[:, 128 * jc : 128 * (jc + 1)],
                            hT[:, 0, tsl], start=True, stop=False,
                        )
                        nc.tensor.matmul(
                            pST, lhsU[0:97, 1, 128 * jc : 128 * (jc + 1)],
                            hT[0:97, 1, tsl], start=False, stop=True,
                        )
                        nc.scalar.activation(pTP[:, jc, half, :], pST, AF.Exp)
                        pT.append(pTP[:, jc, half, :])

                    # j-max batched per group PAIR: one wide max + one reduce
                    if gi % 2 == 1:
                        pmax = mxp.tile([128, 2 * GW * 128], BF16, tag="pmax")
                        nc.vector.tensor_tensor(
                            pmax,
                            pTP[:, 0, :, :].rearrange("p a b -> p (a b)"),
                            pTP[:, 1, :, :].rearrange("p a b -> p (a b)"),
                            op=mybir.AluOpType.max,
                        )
                        pmg = pmx.tile([128, 2 * GW, 128], BF16, tag="mx", name="pmg")
                        for k in range(2 * GW):
                            nc.tensor.transpose(
                                pmg[:, k, :], pmax[:, 128 * k : 128 * (k + 1)],
                                ident_bf,
                            )
                        nc.vector.reduce_max(
                            mcol[:, GW * (gi - 1) : GW * (gi + 1)], pmg, axis=AX.X
                        )

                    # c2q + denominator for the whole group into one psum tile
                    pcg = pcq.tile([128, GW, 256], F32, tag="cq", name="pcg")
                    for k in range(GW):
                        i = GW * gi + k
                        sl = slice(128 * k, 128 * (k + 1))
                        nc.tensor.matmul(
                            pcg[:, k, 0:201], pT[0][:, sl], u2[:, 0, 0:201],
                            start=True, stop=False,
                        )
                        nc.tensor.matmul(
                            pcg[:, k, 0:201], pT[1][:, sl], u2[:, 1, 0:201],
                            start=False, stop=True,
                        )
                        # hw[t] = h.w_h accumulated over both d-chunks; the
                        # pair is emitted back-to-back so the accumulation
                        # group never interleaves with another in this bank
                        nc.tensor.matmul(
                            pq[:, 32 + i : 33 + i], hT[:, 0, 128 * i : 128 * i + 128],
                            wb[:, 0:1], start=True, stop=False,
                        )
                        nc.tensor.matmul(
                            pq[:, 32 + i : 33 + i],
                            hT[0:72, 1, 128 * i : 128 * i + 128],
                            wb[0:72, 1:2], start=False, stop=True,
                        )

                    # one reciprocal for the group's denominators
                    rc = rcp.tile([128, GW], F32, tag="rc")
                    nc.vector.reciprocal(rc, pcg[:, :, 200])
                    gg = ggp.tile([128, GW, 2 * D], BF16, tag="gg")
                    # normalize all GW tiles in one op: rc broadcast along d via
                    # a stride-0 AP (DVE), with an ACT per-tile variant for
                    # load balancing every NORM_ACT_EVERYth group
                    gidx = b * (NT // GW) + gi
                    if NORM_ACT_EVERY and gidx % NORM_ACT_EVERY == NORM_ACT_EVERY - 1:
                        for k in range(GW):
                            nc.scalar.mul(
                                gg[:, k, 0:D], pcg[:, k, 0:D], mul=rc[:, k : k + 1]
                            )
                    else:
                        rca = rc[:, :]
                        rcb = bass.AP(
                            tensor=rca.tensor, offset=rca.offset,
                            ap=[*rca.ap, [0, D]],
                        )
                        nc.vector.tensor_tensor(
                            gg[:, :, 0:D], pcg[:, :, 0:D], rcb,
                            op=mybir.AluOpType.mult,
                        )
                    # h*c2q for the whole group in one op
                    hsb_g = hsb[:, D * GW * gi : D * GW * (gi + 1)].rearrange(
                        "p (k d) -> p k d", d=D
                    )
                    if HC2Q_MOD == 1 or (HC2Q_MOD == 2 and gidx % 2 == 0):
                        nc.gpsimd.tensor_mul(gg[:, :, D : 2 * D], hsb_g, gg[:, :, 0:D])
                    else:
                        nc.vector.tensor_mul(gg[:, :, D : 2 * D], hsb_g, gg[:, :, 0:D])
                    nc.sync.dma_start(
                        out=gb_d[b, :, GW * gi : GW * gi + GW, :],
                        in_=gg,
                    )

                # ---- batch tail: em weights, q2c, broadcast, final product ----
                # em = rowmax_exp * exp(hw), one exp + one mul
                eh16 = smalls.tile([128, NT], F32, tag="eh16")
                nc.scalar.activation(eh16, pq[:, 32:48], AF.Exp)
                nc.gpsimd.tensor_mul(em, mcol, eh16)
                # q2c partials, one column per tile (no open psum groups)
                for i in range(NT):
                    nc.tensor.matmul(
                        pq[:, i : i + 1], hsb[:, D * i : D * i + 128],
                        em[:, i : i + 1], start=True, stop=True,
                    )
                    nc.tensor.matmul(
                        pq[0:72, 16 + i : 17 + i],
                        hsb[:, D * i + 128 : D * i + D],
                        em[:, i : i + 1], start=True, stop=True,
                    )
                nc.tensor.matmul(
                    pq[0:1, 64:80], wb[:, 4:5], em, start=True, stop=True
                )
                rts = smalls.tile([1, 1], F32, tag="rts")
                nc.vector.reduce_sum(rts, pq[0:1, 64:80], axis=AX.X)
                nc.vector.reciprocal(rts, rts)
                rtsc = qtp.tile([128, 1], F32, tag="rtsc")
                nc.gpsimd.partition_broadcast(rtsc, rts)
                q2cT = qtp.tile([128, 2], F32, tag="q2cT")
                nc.vector.reduce_sum(q2cT[:, 0:1], pq[:, 0:16], axis=AX.X)
                nc.vector.reduce_sum(q2cT[0:72, 1:2], pq[0:72, 16:32], axis=AX.X)
                # block 3 feature-major in one fused op per d-chunk:
                # g3T[d,t] = hT[d,t] * q2c_raw[d] * (1/sum em)
                # chunk1 rows 72:128 of q2cT are never written -- compute and
                # store only the valid 0:72 rows for that chunk
                g3t = g3p.tile([128, 2, T], BF16, tag="g3")
                nc.vector.tensor_scalar(
                    g3t[:, 0, :], hT[:, 0, :], q2cT[:, 0:1], rtsc,
                    op0=mybir.AluOpType.mult, op1=mybir.AluOpType.mult,
                )
                nc.vector.tensor_scalar(
                    g3t[0:72, 1, :], hT[0:72, 1, :], q2cT[0:72, 1:2], rtsc[0:72],
                    op0=mybir.AluOpType.mult, op1=mybir.AluOpType.mult,
                )
                getattr(nc, G30_ENG).dma_start(out=g3_d[b, 0], in_=g3t[:, 0, :])
                g31 = "sync" if b == BL - 1 else G31_ENG
                getattr(nc, g31).dma_start(out=g3_d[b, 1, 0:72], in_=g3t[0:72, 1, :])
                if DEBUG_DUMPS:
                    dcp = smalls.tile([128, 80], F32, tag="dcp")
                    nc.scalar.copy(dcp[:, 0:16], pq[:, 0:16])
                    nc.scalar.copy(dcp[:, 32:64], pq[:, 32:64])
                    nc.gpsimd.memset(dcp[:, 16:32], 0.0)
                    nc.gpsimd.memset(dcp[:, 64:80], 0.0)
                    nc.sync.dma_start(out=em_d[b], in_=em)
                    nc.sync.dma_start(out=mc_d[b], in_=mcol)
                    nc.sync.dma_start(out=pq_d[b], in_=dcp)
                    nc.sync.dma_start(out=qt_d[b], in_=q2cT)
                    nc.sync.dma_start(out=rt_d[b], in_=rtsc)
    nc.finalize()
    return nc


def _make_runner(nc):
    """jit-compiled SPMD runner (cached across kernel() calls)."""
    import jax
    from jax.sharding import Mesh, PartitionSpec
    from jax.experimental.shard_map import shard_map
    from concourse import bass2jax
    from concourse.bass2jax import _bass_exec_p, install_neuronx_cc_hook

    install_neuronx_cc_hook()
    partition_name = nc.partition_id_tensor.name if nc.partition_id_tensor else None
    in_names, out_names, out_avals, zero_outs = [], [], [], []
    for alloc in nc.m.functions[0].allocations:
        if not isinstance(alloc, mybir.MemoryLocationSet):
            continue
        name = alloc.memorylocations[0].name
        if alloc.kind == "ExternalInput":
            if name != partition_name:
                in_names.append(name)
        elif alloc.kind == "ExternalOutput":
            out_names.append(name)
            shape = tuple(alloc.tensor_shape)
            dtype = mybir.dt.np(alloc.dtype)
            out_avals.append(jax.core.ShapedArray(shape, dtype))
            zero_outs.append(np.zeros(shape, dtype))
    all_in_names = in_names + out_names
    if partition_name is not None:
        all_in_names = all_in_names + [partition_name]

    def _body(*args):
        operands = list(args)
        if partition_name is not None:
            operands.append(bass2jax.partition_id_tensor())
        return tuple(
            _bass_exec_p.bind(
                *operands,
                out_avals=tuple(out_avals),
                in_names=tuple(all_in_names),
                out_names=tuple(out_names),
                lowering_input_output_aliases=(),
                sim_require_finite=True,
                sim_require_nnan=True,
                nc=nc,
            )
        )

    devices = jax.devices()[:N_CORES]
    mesh = Mesh(np.asarray(devices), ("core",))
    n_all = len(in_names) + len(out_names)
    sharded = jax.jit(
        shard_map(
            _body, mesh=mesh,
            in_specs=(PartitionSpec("core"),) * n_all,
            out_specs=(PartitionSpec("core"),) * len(out_names),
            check_rep=False,
        ),
        keep_unused=True,
    )
    zeros_cat = [np.zeros((N_CORES * z.shape[0], *z.shape[1:]), z.dtype)
                 for z in zero_outs]
    return sharded, in_names, zeros_cat


def prep_inputs(h, u, w_h, w_u, w_hu):
    """Host-side tiling/casting into the device layouts (full-batch views)."""
    hbf = h.astype(BFNP)
    ubf = u.astype(BFNP)
    # token-major: hD[b, p, n*200+d] = h[b, n*128+p, d]
    hD = np.ascontiguousarray(
        hbf.reshape(B, NT, 128, D).transpose(0, 2, 1, 3)
    ).reshape(B, 128, NT * D)
    # feature-major with ones row at chunk1 row 72
    ht = hbf.transpose(0, 2, 1)  # [B, D, T]
    hTD = np.zeros((B, 2, 128, T), dtype=BFNP)
    hTD[:, 0, :, :] = ht[:, 0:128, :]
    hTD[:, 1, 0:72, :] = ht[:, 128:D, :]
    hTD[:, 1, 96, :] = BFNP(1.0)   # meets the uw row (96) of lhsU1
    # u token-major (j on partitions) + ones column at 200
    u2D = np.zeros((B, 128, 2, 208), dtype=BFNP)
    u2D[:, :, 0, 0:D] = ubf[:, 0:128, :]
    u2D[:, :, 1, 0:D] = ubf[:, 128:J, :]
    u2D[:, :, :, D] = BFNP(1.0)
    # u feature-major for the lhsT build
    ut = ubf.transpose(0, 2, 1)  # [B, D, J]
    uTD = np.zeros((B, 2, 128, 256), dtype=BFNP)
    uTD[:, 0, :, 0:J] = ut[:, 0:128, :]
    uTD[:, 1, 0:72, 0:J] = ut[:, 128:D, :]
    # weight columns
    wbf = np.zeros((128, 6), dtype=BFNP)
    wbf[:, 0] = w_h[0:128].astype(BFNP)
    wbf[0:72, 1] = w_h[128:D].astype(BFNP)
    wbf[:, 2] = w_u[0:128].astype(BFNP)
    wbf[0:72, 3] = w_u[128:D].astype(BFNP)
    wbf[:, 4] = BFNP(1.0)
    wf = np.zeros((128, 2), dtype=np.float32)
    wf[:, 0] = w_hu[0:128]
    wf[0:72, 1] = w_hu[128:D]
    return hD, hTD, u2D, uTD, wbf, wf


def kernel(**inputs):
    h = np.ascontiguousarray(np.asarray(inputs["h"], dtype=np.float32))
    u = np.ascontiguousarray(np.asarray(inputs["u"], dtype=np.float32))
    w_h = np.asarray(inputs["w_h"], dtype=np.float32)
    w_u = np.asarray(inputs["w_u"], dtype=np.float32)
    w_hu = np.asarray(inputs["w_hu"], dtype=np.float32)

    if "runner" not in _cache:
        _cache["nc"] = build_nc()
        _cache["runner"] = _make_runner(_cache["nc"])
    sharded, in_names, zeros_cat = _cache["runner"]

    hD, hTD, u2D, uTD, wbf, wf = prep_inputs(h, u, w_h, w_u, w_hu)
    full = {
        "hD": hD, "hTD": hTD, "u2D": u2D, "uTD": uTD,
        "wbf": np.concatenate([wbf] * N_CORES, axis=0),
        "wf": np.concatenate([wf] * N_CORES, axis=0),
    }
    args = [full[name] for name in in_names] + zeros_cat
    out = sharded(*args)
    outs = {name: np.asarray(o) for name, o in
            zip([n for n in ("gbD", "g3D")], out)}
    gb = outs["gbD"].reshape(B, 128, NT, 2 * D)
    g3 = outs["g3D"].reshape(B, 2, 128, T)

    g = np.empty((B, T, 4 * D), dtype=np.float32)
    g[:, :, 0:D] = h
    g[:, :, D : 3 * D] = (
        gb.transpose(0, 2, 1, 3).reshape(B, T, 2 * D).astype(np.float32)
    )
    blk3 = np.concatenate([g3[:, 0, :, :], g3[:, 1, 0:72, :]], axis=1)
    g[:, :, 3 * D : 4 * D] = blk3.transpose(0, 2, 1).astype(np.float32)
    return g
